# revision 11
# baseline (speedup 1.0000x reference)
"""BiDAF attention + masked max-pool + classifier kernel for Trainium2.

Reference computation (per batch b):
  S = H @ W_attn @ U^T                       (P, Q)
  c2q = softmax_q(S) @ U                     (P, D)
  b_attn = softmax_p(max_q S)                (P,)
  q2c = b_attn @ H                           (D,)
  G_M = [H; c2q; H*c2q; H*q2c; M]            (P, 5D)
  pooled = max over non-pad p of G_M         (5D,)
  out = pooled @ W_cls                       (2,)

Sharding: data-parallel over batch. B=32 -> 8 cores x 4 batches.

Device-side notes:
  * S is computed as H @ Wu with Wu = W_attn @ U^T (one matmul per
    128-row chunk of H, with H^T produced on-chip by PE transposes).
  * softmax_q skips the max-subtraction: |S| <= ~70 so exp(S) is in
    fp32 range; probs are normalized with 1/Z (Z from the ACT-exp
    accumulator).
  * b_attn = exp(m - g) / sum(exp(m - g)) where m = max_q S (rowmax) and
    g the global max; q2c is a chain of 32 accumulating matmuls with the
    natural-layout H chunks as stationary weights.
  * The pad-mask enters only via the max-pool.  A -1e30 bias row is
    broadcast across partitions (GPSIMD partition_broadcast) and folded
    into the pooling reductions with fused tensor_tensor_reduce ops:
      maxH  = reduce_max(H^T + mask)
      -minH = reduce_max(mask - H^T)
      maxC  = reduce_max(c2q^T + mask)
      maxP  = reduce_max(H^T * c2q^T + mask)
    max-pool of H*q2c is reconstructed from maxH/minH and the sign of
    q2c (q2c is constant over p).
  * M is pooled in natural layout on GPSIMD with fused
    scalar_tensor_tensor (mask-add + running max), then reduced at
    batch end (free-axis reduce + PE transpose + reduce).
"""

import sys

for _p in ("/opt/trn_rl_repo", "/opt/trn_rl_repo/concourse"):
    if _p not in sys.path:
        sys.path.insert(0, _p)

from contextlib import ExitStack

import numpy as np

import concourse.bass as bass
import concourse.tile as tile
from concourse import bacc, masks, mybir
from concourse.bass_utils import run_bass_kernel_spmd

F32 = mybir.dt.float32
BF16 = mybir.dt.bfloat16
ALU = mybir.AluOpType
AF = mybir.ActivationFunctionType

N_CORES = 8
B, P, Q, D = 32, 4096, 64, 128
B_CORE = B // N_CORES          # 4 batches per core
NB = 8                         # p-blocks per batch (of 512)
BLK = P // NB                  # 512
CH = BLK // 128                # 4 chunks of 128 per block
NEG = -1.0e30
NEG_INIT = -3.0e38


def build_program():
    nc = bacc.Bacc("TRN2", target_bir_lowering=False, debug=False,
                   num_devices=N_CORES)

    h_ext = nc.dram_tensor("h", [B_CORE, P, D], F32, kind="ExternalInput").ap()
    m_ext = nc.dram_tensor("m", [B_CORE, P, D], F32, kind="ExternalInput").ap()
    u_ext = nc.dram_tensor("u", [B_CORE, Q, D], F32, kind="ExternalInput").ap()
    w_ext = nc.dram_tensor("w", [D, D], F32, kind="ExternalInput").ap()
    wcls_ext = nc.dram_tensor("wcls", [5 * D, 2], F32, kind="ExternalInput").ap()
    # mask bias rows: -1e30 at pad positions, 0 elsewhere
    mrow_ext = nc.dram_tensor("mrow", [B_CORE, P], F32, kind="ExternalInput").ap()
    mrow16_ext = nc.dram_tensor("mrow16", [B_CORE, P], BF16,
                                kind="ExternalInput").ap()
    mrow16p_ext = nc.dram_tensor("mrow16p", [B_CORE, P], BF16,
                                 kind="ExternalInput").ap()
    out_ext = nc.dram_tensor("out", [B_CORE, 2], F32, kind="ExternalOutput").ap()

    with tile.TileContext(nc) as tc, ExitStack() as ctx:
        pool1 = ctx.enter_context(tc.tile_pool(name="const", bufs=1))
        poolb = ctx.enter_context(tc.tile_pool(name="batch", bufs=2))
        poolk = ctx.enter_context(tc.tile_pool(name="blk", bufs=3))
        poolw = ctx.enter_context(tc.tile_pool(name="work", bufs=2))
        psA = ctx.enter_context(tc.tile_pool(name="psA", bufs=2, space="PSUM"))
        psB = ctx.enter_context(tc.tile_pool(name="psB", bufs=2, space="PSUM"))
        psC = ctx.enter_context(tc.tile_pool(name="psC", bufs=1, space="PSUM"))
        psD = ctx.enter_context(tc.tile_pool(name="psD", bufs=1, space="PSUM"))
        psE = ctx.enter_context(tc.tile_pool(name="psE", bufs=1, space="PSUM"))
        psF = ctx.enter_context(tc.tile_pool(name="psF", bufs=1, space="PSUM"))

        # ---- once-per-kernel constants ----
        ident32 = pool1.tile([128, 128], F32)
        masks.make_identity(nc, ident32[:])
        ident16 = pool1.tile([128, 128], BF16)
        masks.make_identity(nc, ident16[:])
        onescol = pool1.tile([128, 1], F32)
        nc.vector.memset(onescol[:], 1.0)
        ones16 = pool1.tile([1, 128], BF16)
        nc.vector.memset(ones16[:], 1.0)

        w_sb = pool1.tile([D, D], F32)
        nc.sync.dma_start(w_sb[:], w_ext[:])
        wcls_sb = pool1.tile([D, 5, 2], F32)
        nc.sync.dma_start(wcls_sb[:], wcls_ext.rearrange("(k d) o -> d k o", k=5))

        wt_ps = psC.tile([D, D], F32, tag="small")
        nc.tensor.transpose(wt_ps[:], w_sb[:], ident32[:])
        wt_sb = pool1.tile([D, D], F32)
        nc.scalar.copy(wt_sb[:], wt_ps[:])

        for b in range(B_CORE):
            # ---- per-batch prep ----
            hn = poolb.tile([128, P // 128, D], F32, tag="hn")
            nc.sync.dma_start(hn[:], h_ext[b].rearrange("(c l) d -> l c d", l=128))

            u_sb = poolb.tile([Q, D], F32, tag="u")
            nc.sync.dma_start(u_sb[:], u_ext[b])
            u16 = poolb.tile([Q, D], BF16, tag="u16")
            nc.scalar.copy(u16[:], u_sb[:])

            ut_ps = psC.tile([D, Q], F32, tag="small")
            nc.tensor.transpose(ut_ps[:], u_sb[:], ident32[:Q, :Q])
            ut_sb = poolb.tile([D, Q], F32, tag="ut")
            nc.scalar.copy(ut_sb[:], ut_ps[:])

            wu_ps = psC.tile([D, Q], F32, tag="small")
            nc.tensor.matmul(wu_ps[:], lhsT=wt_sb[:], rhs=ut_sb[:],
                             start=True, stop=True)
            wu_sb = poolb.tile([D, Q], F32, tag="wu")
            nc.scalar.copy(wu_sb[:], wu_ps[:])

            mrow = poolb.tile([1, P], F32, tag="mrow")
            nc.sync.dma_start(mrow[:], mrow_ext[b, None, :])
            mrow16 = poolb.tile([1, P], BF16, tag="mrow16")
            nc.sync.dma_start(mrow16[:], mrow16_ext[b, None, :])
            mrow16p = poolb.tile([1, P], BF16, tag="mrow16p")
            nc.sync.dma_start(mrow16p[:], mrow16p_ext[b, None, :])

            # per-batch stats / accumulators
            mx = poolb.tile([128, P // 128], F32, tag="mx")          # rowmax of S
            zc = poolb.tile([128, P // 128], F32, tag="zc")          # rowsum exp
            rz = poolb.tile([128, P // 128], F32, tag="rz")          # 1/Z
            maxh_c = poolb.tile([128, NB], F32, tag="maxh")
            minh_c = poolb.tile([128, NB], F32, tag="minh")
            maxc_c = poolb.tile([128, NB], F32, tag="maxc")
            maxp_c = poolb.tile([128, NB], F32, tag="maxp")
            macc = poolb.tile([128, CH, D], F32, tag="macc")
            nc.vector.memset(macc[:], NEG_INIT)

            for blk in range(NB):
                p0 = blk * BLK
                # mask row for this block, broadcast across partitions
                mb = poolk.tile([128, BLK], F32, tag="mb")
                nc.gpsimd.partition_broadcast(mb[:], mrow[:, p0:p0 + BLK])

                # M block (natural layout) + masked running max on GPSIMD
                mn = poolk.tile([128, CH, D], F32, tag="mn")
                nc.sync.dma_start(
                    mn[:], m_ext[b, p0:p0 + BLK].rearrange("(c l) d -> l c d", l=128))
                nc.vector.tensor_tensor(out=macc[:], in0=mn[:], in1=macc[:],
                                        op=ALU.max)

                # H^T for this block via PE transposes
                ht_ps = psA.tile([128, BLK], F32, tag="ht_ps")
                for c in range(CH):
                    nc.tensor.matmul(ht_ps[:, c * 128:(c + 1) * 128],
                                     lhsT=hn[:, blk * CH + c, :], rhs=ident32[:],
                                     is_transpose=True, start=(c == 0),
                                     stop=(c == CH - 1), skip_group_check=True)
                ht_sb = poolk.tile([128, BLK], F32, tag="ht_sb")
                nc.scalar.copy(ht_sb[:], ht_ps[:])

                # S chunks: [p=128, q=64] = (H^T chunk)^T @ Wu
                s_ps = psB.tile([128, CH, Q], F32, tag="s_ps")
                for c in range(CH):
                    nc.tensor.matmul(s_ps[:, c, :],
                                     lhsT=ht_sb[:, c * 128:(c + 1) * 128],
                                     rhs=wu_sb[:], start=(c == 0),
                                     stop=(c == CH - 1), skip_group_check=True)

                # rowmax (for b_attn)
                nc.vector.reduce_max(mx[:, blk * CH:(blk + 1) * CH], s_ps[:],
                                     axis=mybir.AxisListType.X)

                # exp (no max subtraction), one ACT op, bf16 out
                probs = poolk.tile([128, CH, Q], BF16, tag="probs")
                nc.scalar.activation(probs[:], s_ps[:], AF.Exp)
                nc.vector.reduce_sum(zc[:, blk * CH:(blk + 1) * CH, None],
                                     probs[:], axis=mybir.AxisListType.X)
                nc.vector.reciprocal(rz[:, blk * CH:(blk + 1) * CH],
                                     zc[:, blk * CH:(blk + 1) * CH])
                nc.vector.tensor_tensor(
                    out=probs[:], in0=probs[:],
                    in1=rz[:, blk * CH:(blk + 1) * CH, None].broadcast_to(
                        (128, CH, Q)),
                    op=ALU.mult)

                # probs^T via PE transposes -> [q=64, p=512]
                pt_ps = psD.tile([Q, CH, 128], BF16, tag="pt_ps")
                for c in range(CH):
                    nc.tensor.matmul(pt_ps[:, c, :], lhsT=probs[:, c, :],
                                     rhs=ident16[:], is_transpose=True,
                                     start=(c == 0), stop=(c == CH - 1),
                                     skip_group_check=True)
                pt_sb = poolk.tile([Q, CH * 128], BF16, tag="pt_sb")
                nc.scalar.copy(pt_sb[:], pt_ps[:].rearrange("q c l -> q (c l)"))

                # c2q^T = U^T(bf16) @ probs^T : [d=128, p=512]
                c2q_ps = psE.tile([D, BLK], F32, tag="c2q_ps")
                nc.tensor.matmul(c2q_ps[:], lhsT=u16[:], rhs=pt_sb[:],
                                 start=True, stop=True)
                c2q_sb = poolk.tile([D, BLK], F32, tag="c2q_sb")
                nc.scalar.copy(c2q_sb[:], c2q_ps[:])

                # H*c2q product stream (GPSIMD, SBUF only), then masked
                prod = poolk.tile([128, BLK], F32, tag="prod")
                nc.gpsimd.tensor_tensor(out=prod[:], in0=ht_sb[:], in1=c2q_sb[:],
                                        op=ALU.mult)
                nc.gpsimd.tensor_tensor(out=prod[:], in0=prod[:], in1=mb[:],
                                        op=ALU.add)
                nc.vector.reduce_max(maxp_c[:, blk, None], prod[:],
                                     axis=mybir.AxisListType.X)

                # masked max/min of H: accumulate mask rows into PSUM via
                # k=1 matmuls, reduce between them
                nc.tensor.matmul(ht_ps[:], lhsT=ones16[:], rhs=mrow16[:, p0:p0 + BLK],
                                 start=False, stop=True, skip_group_check=True)
                nc.vector.reduce_max(maxh_c[:, blk, None], ht_ps[:],
                                     axis=mybir.AxisListType.X)
                nc.tensor.matmul(ht_ps[:], lhsT=ones16[:], rhs=mrow16p[:, p0:p0 + BLK],
                                 start=False, stop=True, skip_group_check=True)
                nc.vector.tensor_reduce(minh_c[:, blk, None], ht_ps[:],
                                        axis=mybir.AxisListType.X, op=ALU.min)

                # masked max of c2q: same PSUM trick
                nc.tensor.matmul(c2q_ps[:], lhsT=ones16[:], rhs=mrow16[:, p0:p0 + BLK],
                                 start=False, stop=True, skip_group_check=True)
                nc.vector.reduce_max(maxc_c[:, blk, None], c2q_ps[:],
                                     axis=mybir.AxisListType.X)

            # ---- batch epilogue ----
            # global rowmax g over all p
            m1 = poolb.tile([128, 1], F32, tag="m1")
            nc.vector.reduce_max(m1[:], mx[:], axis=mybir.AxisListType.X)
            mt_ps = psC.tile([1, 128], F32, tag="small")
            nc.tensor.transpose(mt_ps[:], m1[:], ident32[:])
            g1 = poolb.tile([1, 1], F32, tag="g1")
            nc.vector.reduce_max(g1[:], mt_ps[:], axis=mybir.AxisListType.X)
            negg = poolb.tile([1, 1], F32, tag="negg")
            nc.vector.tensor_scalar_mul(negg[:], g1[:], -1.0)
            neggb = poolb.tile([128, 1], F32, tag="neggb")
            nc.gpsimd.partition_broadcast(neggb[:], negg[:])

            bexp = poolb.tile([128, P // 128], F32, tag="bexp")
            nc.scalar.activation(bexp[:], mx[:], AF.Exp, bias=neggb[:, 0, None])

            # q2c (unnormalized): sum_p exp(m_p - g) * H[p, :]
            q2c_ps = psF.tile([D, 1], F32, tag="q2c_ps")
            for c in range(P // 128):
                nc.tensor.matmul(q2c_ps[:], lhsT=hn[:, c, :],
                                 rhs=bexp[:, c, None],
                                 start=(c == 0), stop=(c == P // 128 - 1))

            # Zb = sum_p exp(m_p - g)
            zrow_ps = psC.tile([1, P // 128], F32, tag="small")
            nc.tensor.matmul(zrow_ps[:], lhsT=onescol[:], rhs=bexp[:],
                             start=True, stop=True)
            zb = poolb.tile([1, 1], F32, tag="zb")
            nc.vector.reduce_sum(zb[:], zrow_ps[:], axis=mybir.AxisListType.X)
            rzb = poolb.tile([1, 1], F32, tag="rzb")
            nc.vector.reciprocal(rzb[:], zb[:])
            rzbb = poolb.tile([128, 1], F32, tag="rzbb")
            nc.gpsimd.partition_broadcast(rzbb[:], rzb[:])

            q2c = poolb.tile([D, 1], F32, tag="q2c")
            nc.vector.tensor_scalar_mul(q2c[:], q2c_ps[:], rzbb[:, 0, None])

            # pooled columns [d, 5]: [maxH, maxC, maxP, maxHq2c, maxM]
            pooled = poolb.tile([128, 5], F32, tag="pooled")
            nc.vector.reduce_max(pooled[:, 0, None], maxh_c[:],
                                 axis=mybir.AxisListType.X)
            nc.vector.reduce_max(pooled[:, 1, None], maxc_c[:],
                                 axis=mybir.AxisListType.X)
            nc.vector.reduce_max(pooled[:, 2, None], maxp_c[:],
                                 axis=mybir.AxisListType.X)

            # max over valid p of H*q2c from maxH/minH and q2c sign
            nm = poolb.tile([128, 1], F32, tag="nm")
            nc.vector.tensor_reduce(nm[:], minh_c[:], axis=mybir.AxisListType.X,
                                    op=ALU.min)
            t1 = poolb.tile([128, 1], F32, tag="t1")
            nc.vector.tensor_tensor(out=t1[:], in0=q2c[:],
                                    in1=pooled[:, 0, None], op=ALU.mult)
            t2 = poolb.tile([128, 1], F32, tag="t2")
            nc.vector.tensor_tensor(out=t2[:], in0=q2c[:], in1=nm[:], op=ALU.mult)
            nc.vector.tensor_tensor(out=pooled[:, 3, None], in0=t1[:], in1=t2[:],
                                    op=ALU.max)

            # M: fold macc chunks, transpose, reduce over lanes
            mfold = poolb.tile([128, D], F32, tag="mfold")
            nc.vector.reduce_max(
                mfold[:], macc[:].rearrange("l c d -> l d c"),
                axis=mybir.AxisListType.X)
            mt2_ps = psC.tile([D, 128], F32, tag="small")
            nc.tensor.transpose(mt2_ps[:], mfold[:], ident32[:])
            nc.vector.reduce_max(pooled[:, 4, None], mt2_ps[:],
                                 axis=mybir.AxisListType.X)

            # final classifier: out[1,2] = sum_k pooled[:,k]^T @ Wcls[k]
            out_ps = psC.tile([1, 2], F32, tag="small")
            for k in range(5):
                nc.tensor.matmul(out_ps[:], lhsT=pooled[:, k, None],
                                 rhs=wcls_sb[:, k, :],
                                 start=(k == 0), stop=(k == 4))
            out_sb = poolb.tile([1, 2], F32, tag="out_sb")
            nc.scalar.copy(out_sb[:], out_ps[:])
            nc.sync.dma_start(out_ext[b, None, :], out_sb[:])

    nc.compile()
    return nc


_CACHED_NC = None


def _get_program():
    global _CACHED_NC
    if _CACHED_NC is None:
        _CACHED_NC = build_program()
    return _CACHED_NC


def make_in_maps(tensor_H, tensor_U, M, sentence_word_rep, W_attn, W_cls):
    tensor_H = np.ascontiguousarray(np.asarray(tensor_H, dtype=np.float32))
    tensor_U = np.ascontiguousarray(np.asarray(tensor_U, dtype=np.float32))
    M = np.ascontiguousarray(np.asarray(M, dtype=np.float32))
    W_attn = np.ascontiguousarray(np.asarray(W_attn, dtype=np.float32))
    W_cls = np.ascontiguousarray(np.asarray(W_cls, dtype=np.float32))
    swr = np.asarray(sentence_word_rep)

    import ml_dtypes
    bias = np.where(swr == 0, np.float32(NEG), np.float32(0.0)).astype(np.float32)
    bias16 = bias.astype(ml_dtypes.bfloat16)
    M = M.copy()
    M[np.asarray(swr) == 0] = np.float32(NEG)
    bias16p = (-2.0 * bias).astype(ml_dtypes.bfloat16)

    in_maps = []
    for core in range(N_CORES):
        sl = slice(core * B_CORE, (core + 1) * B_CORE)
        in_maps.append({
            "h": tensor_H[sl],
            "m": M[sl],
            "u": tensor_U[sl],
            "w": W_attn,
            "wcls": W_cls,
            "mrow": np.ascontiguousarray(bias[sl]),
            "mrow16": np.ascontiguousarray(bias16[sl]),
            "mrow16p": np.ascontiguousarray(bias16p[sl]),
        })
    return in_maps


def kernel(tensor_H, tensor_U, M, sentence_word_rep, W_attn, W_cls):
    nc = _get_program()
    in_maps = make_in_maps(tensor_H, tensor_U, M, sentence_word_rep,
                           W_attn, W_cls)
    res = run_bass_kernel_spmd(nc, in_maps, list(range(N_CORES)))
    out = np.concatenate([res.results[i]["out"] for i in range(N_CORES)], axis=0)
    return out.astype(np.float32)


# revision 15
# speedup vs baseline: 1.0369x; 1.0369x over previous
"""BiDAF attention + masked max-pool + classifier kernel for Trainium2.

Reference computation (per batch b):
  S = H @ W_attn @ U^T                       (P, Q)
  c2q = softmax_q(S) @ U                     (P, D)
  b_attn = softmax_p(max_q S)                (P,)
  q2c = b_attn @ H                           (D,)
  G_M = [H; c2q; H*c2q; H*q2c; M]            (P, 5D)
  pooled = max over non-pad p of G_M         (5D,)
  out = pooled @ W_cls                       (2,)

Sharding: data-parallel over batch. B=32 -> 8 cores x 4 batches.

Device-side notes:
  * S is computed as H @ Wu with Wu = W_attn @ U^T (one matmul per
    128-row chunk of H, with H^T produced on-chip by PE transposes).
  * softmax_q skips the max-subtraction: |S| <= ~70 so exp(S) is in
    fp32 range; probs are normalized with 1/Z (Z from the ACT-exp
    accumulator).
  * b_attn = exp(m - g) / sum(exp(m - g)) where m = max_q S (rowmax) and
    g the global max; q2c is a chain of 32 accumulating matmuls with the
    natural-layout H chunks as stationary weights.
  * The pad-mask enters only via the max-pool.  A -1e30 bias row is
    broadcast across partitions (GPSIMD partition_broadcast) and folded
    into the pooling reductions with fused tensor_tensor_reduce ops:
      maxH  = reduce_max(H^T + mask)
      -minH = reduce_max(mask - H^T)
      maxC  = reduce_max(c2q^T + mask)
      maxP  = reduce_max(H^T * c2q^T + mask)
    max-pool of H*q2c is reconstructed from maxH/minH and the sign of
    q2c (q2c is constant over p).
  * M is pooled in natural layout on GPSIMD with fused
    scalar_tensor_tensor (mask-add + running max), then reduced at
    batch end (free-axis reduce + PE transpose + reduce).
"""

import sys

for _p in ("/opt/trn_rl_repo", "/opt/trn_rl_repo/concourse"):
    if _p not in sys.path:
        sys.path.insert(0, _p)

from contextlib import ExitStack

import numpy as np

import concourse.bass as bass
import concourse.tile as tile
from concourse import bacc, masks, mybir
from concourse.bass_utils import run_bass_kernel_spmd

F32 = mybir.dt.float32
BF16 = mybir.dt.bfloat16
ALU = mybir.AluOpType
AF = mybir.ActivationFunctionType

N_CORES = 8
B, P, Q, D = 32, 4096, 64, 128
B_CORE = B // N_CORES          # 4 batches per core
NB = 8                         # p-blocks per batch (of 512)
BLK = P // NB                  # 512
CH = BLK // 128                # 4 chunks of 128 per block
NEG = -1.0e30
NEG_INIT = -3.0e38


def build_program():
    nc = bacc.Bacc("TRN2", target_bir_lowering=False, debug=False,
                   num_devices=N_CORES)

    h_ext = nc.dram_tensor("h", [B_CORE, P, D], F32, kind="ExternalInput").ap()
    m_ext = nc.dram_tensor("m", [B_CORE, P, D], F32, kind="ExternalInput").ap()
    u_ext = nc.dram_tensor("u", [B_CORE, Q, D], F32, kind="ExternalInput").ap()
    w_ext = nc.dram_tensor("w", [D, D], F32, kind="ExternalInput").ap()
    wcls_ext = nc.dram_tensor("wcls", [5 * D, 2], F32, kind="ExternalInput").ap()
    # mask bias rows: -1e30 at pad positions, 0 elsewhere
    mrow_ext = nc.dram_tensor("mrow", [B_CORE, P], F32, kind="ExternalInput").ap()
    mrow16_ext = nc.dram_tensor("mrow16", [B_CORE, P], BF16,
                                kind="ExternalInput").ap()
    mrow16p_ext = nc.dram_tensor("mrow16p", [B_CORE, P], BF16,
                                 kind="ExternalInput").ap()
    out_ext = nc.dram_tensor("out", [B_CORE, 2], F32, kind="ExternalOutput").ap()

    with tile.TileContext(nc) as tc, ExitStack() as ctx:
        pool1 = ctx.enter_context(tc.tile_pool(name="const", bufs=1))
        poolb = ctx.enter_context(tc.tile_pool(name="batch", bufs=2))
        poolk = ctx.enter_context(tc.tile_pool(name="blk", bufs=4))
        poolw = ctx.enter_context(tc.tile_pool(name="work", bufs=3))
        psA = ctx.enter_context(tc.tile_pool(name="psA", bufs=2, space="PSUM"))
        psB = ctx.enter_context(tc.tile_pool(name="psB", bufs=2, space="PSUM"))
        psC = ctx.enter_context(tc.tile_pool(name="psC", bufs=1, space="PSUM"))
        psD = ctx.enter_context(tc.tile_pool(name="psD", bufs=1, space="PSUM"))
        psE = ctx.enter_context(tc.tile_pool(name="psE", bufs=1, space="PSUM"))
        psF = ctx.enter_context(tc.tile_pool(name="psF", bufs=1, space="PSUM"))

        # ---- once-per-kernel constants ----
        ident32 = pool1.tile([128, 128], F32)
        masks.make_identity(nc, ident32[:])
        ident16 = pool1.tile([128, 128], BF16)
        masks.make_identity(nc, ident16[:])
        onescol = pool1.tile([128, 1], F32)
        nc.vector.memset(onescol[:], 1.0)
        ones16 = pool1.tile([1, 128], BF16)
        nc.vector.memset(ones16[:], 1.0)

        w_sb = pool1.tile([D, D], F32)
        nc.sync.dma_start(w_sb[:], w_ext[:])
        wcls_sb = pool1.tile([D, 5, 2], F32)
        nc.sync.dma_start(wcls_sb[:], wcls_ext.rearrange("(k d) o -> d k o", k=5))

        wt_ps = psC.tile([D, D], F32, tag="small")
        nc.tensor.transpose(wt_ps[:], w_sb[:], ident32[:])
        wt_sb = pool1.tile([D, D], F32)
        nc.scalar.copy(wt_sb[:], wt_ps[:])

        for b in range(B_CORE):
            # ---- per-batch prep ----
            hn = poolb.tile([128, P // 128, D], F32, tag="hn")
            nc.sync.dma_start(hn[:], h_ext[b].rearrange("(c l) d -> l c d", l=128))

            u_sb = poolb.tile([Q, D], F32, tag="u")
            nc.sync.dma_start(u_sb[:], u_ext[b])
            u16 = poolb.tile([Q, D], BF16, tag="u16")
            nc.scalar.copy(u16[:], u_sb[:])

            ut_ps = psC.tile([D, Q], F32, tag="small")
            nc.tensor.transpose(ut_ps[:], u_sb[:], ident32[:Q, :Q])
            ut_sb = poolb.tile([D, Q], F32, tag="ut")
            nc.scalar.copy(ut_sb[:], ut_ps[:])

            wu_ps = psC.tile([D, Q], F32, tag="small")
            nc.tensor.matmul(wu_ps[:], lhsT=wt_sb[:], rhs=ut_sb[:],
                             start=True, stop=True)
            wu_sb = poolb.tile([D, Q], F32, tag="wu")
            nc.scalar.copy(wu_sb[:], wu_ps[:])

            mrow = poolb.tile([1, P], F32, tag="mrow")
            nc.sync.dma_start(mrow[:], mrow_ext[b, None, :])
            mrow16 = poolb.tile([1, P], BF16, tag="mrow16")
            nc.sync.dma_start(mrow16[:], mrow16_ext[b, None, :])
            mrow16p = poolb.tile([1, P], BF16, tag="mrow16p")
            nc.sync.dma_start(mrow16p[:], mrow16p_ext[b, None, :])

            # per-batch stats / accumulators
            mx = poolb.tile([128, P // 128], F32, tag="mx")          # rowmax of S
            zc = poolb.tile([128, P // 128], F32, tag="zc")          # rowsum exp
            rz = poolb.tile([128, P // 128], F32, tag="rz")          # 1/Z
            maxh_c = poolb.tile([128, NB], F32, tag="maxh")
            minh_c = poolb.tile([128, NB], F32, tag="minh")
            maxc_c = poolb.tile([128, NB], F32, tag="maxc")
            maxp_c = poolb.tile([128, NB], F32, tag="maxp")
            macc = poolb.tile([128, CH, D], F32, tag="macc")
            nc.vector.memset(macc[:], NEG_INIT)

            for blk in range(NB):
                p0 = blk * BLK
                # mask row for this block, broadcast across partitions
                mb = poolk.tile([128, BLK], F32, tag="mb")
                nc.gpsimd.partition_broadcast(mb[:], mrow[:, p0:p0 + BLK])

                # M block (natural layout) + masked running max on GPSIMD
                mn = poolk.tile([128, CH, D], F32, tag="mn")
                nc.sync.dma_start(
                    mn[:], m_ext[b, p0:p0 + BLK].rearrange("(c l) d -> l c d", l=128))
                nc.vector.tensor_tensor(out=macc[:], in0=mn[:], in1=macc[:],
                                        op=ALU.max)

                # H^T for this block via PE transposes
                ht_ps = psA.tile([128, BLK], F32, tag="ht_ps")
                for c in range(CH):
                    nc.tensor.matmul(ht_ps[:, c * 128:(c + 1) * 128],
                                     lhsT=hn[:, blk * CH + c, :], rhs=ident32[:],
                                     is_transpose=True, start=(c == 0),
                                     stop=(c == CH - 1), skip_group_check=True)
                ht_sb = poolk.tile([128, BLK], F32, tag="ht_sb")
                nc.scalar.copy(ht_sb[:], ht_ps[:])

                # S chunks: [p=128, q=64] = (H^T chunk)^T @ Wu
                s_ps = psB.tile([128, CH, Q], F32, tag="s_ps")
                for c in range(CH):
                    nc.tensor.matmul(s_ps[:, c, :],
                                     lhsT=ht_sb[:, c * 128:(c + 1) * 128],
                                     rhs=wu_sb[:], start=(c == 0),
                                     stop=(c == CH - 1), skip_group_check=True)

                # rowmax (for b_attn)
                nc.vector.reduce_max(mx[:, blk * CH:(blk + 1) * CH], s_ps[:],
                                     axis=mybir.AxisListType.X)

                # exp (no max subtraction), one ACT op, bf16 out
                probs = poolk.tile([128, CH, Q], BF16, tag="probs")
                nc.scalar.activation(probs[:], s_ps[:], AF.Exp)
                nc.vector.reduce_sum(zc[:, blk * CH:(blk + 1) * CH, None],
                                     probs[:], axis=mybir.AxisListType.X)
                nc.vector.reciprocal(rz[:, blk * CH:(blk + 1) * CH],
                                     zc[:, blk * CH:(blk + 1) * CH])
                nc.vector.tensor_tensor(
                    out=probs[:], in0=probs[:],
                    in1=rz[:, blk * CH:(blk + 1) * CH, None].broadcast_to(
                        (128, CH, Q)),
                    op=ALU.mult)

                # probs^T via PE transposes -> [q=64, p=512]
                pt_ps = psD.tile([Q, CH, 128], BF16, tag="pt_ps")
                for c in range(CH):
                    nc.tensor.matmul(pt_ps[:, c, :], lhsT=probs[:, c, :],
                                     rhs=ident16[:], is_transpose=True,
                                     start=(c == 0), stop=(c == CH - 1),
                                     skip_group_check=True)
                pt_sb = poolk.tile([Q, CH * 128], BF16, tag="pt_sb")
                nc.scalar.copy(pt_sb[:], pt_ps[:].rearrange("q c l -> q (c l)"))

                # c2q^T = U^T(bf16) @ probs^T : [d=128, p=512]
                c2q_ps = psE.tile([D, BLK], F32, tag="c2q_ps")
                nc.tensor.matmul(c2q_ps[:], lhsT=u16[:], rhs=pt_sb[:],
                                 start=True, stop=True)
                c2q_sb = poolk.tile([D, BLK], F32, tag="c2q_sb")
                nc.scalar.copy(c2q_sb[:], c2q_ps[:])

                # H*c2q product stream (GPSIMD, SBUF only), then masked
                prod = poolk.tile([128, BLK], F32, tag="prod")
                nc.gpsimd.tensor_tensor(out=prod[:], in0=ht_sb[:], in1=c2q_sb[:],
                                        op=ALU.mult)
                nc.gpsimd.tensor_tensor(out=prod[:], in0=prod[:], in1=mb[:],
                                        op=ALU.add)
                nc.vector.reduce_max(maxp_c[:, blk, None], prod[:],
                                     axis=mybir.AxisListType.X)

                # masked max/min of H: accumulate mask rows into PSUM via
                # k=1 matmuls, reduce between them
                nc.tensor.matmul(ht_ps[:], lhsT=ones16[:], rhs=mrow16[:, p0:p0 + BLK],
                                 start=False, stop=True, skip_group_check=True)
                nc.vector.reduce_max(maxh_c[:, blk, None], ht_ps[:],
                                     axis=mybir.AxisListType.X)
                nc.tensor.matmul(ht_ps[:], lhsT=ones16[:], rhs=mrow16p[:, p0:p0 + BLK],
                                 start=False, stop=True, skip_group_check=True)
                nc.vector.tensor_reduce(minh_c[:, blk, None], ht_ps[:],
                                        axis=mybir.AxisListType.X, op=ALU.min)

                # masked max of c2q: same PSUM trick
                nc.tensor.matmul(c2q_ps[:], lhsT=ones16[:], rhs=mrow16[:, p0:p0 + BLK],
                                 start=False, stop=True, skip_group_check=True)
                nc.vector.reduce_max(maxc_c[:, blk, None], c2q_ps[:],
                                     axis=mybir.AxisListType.X)

            # ---- batch epilogue ----
            # global rowmax g over all p
            m1 = poolb.tile([128, 1], F32, tag="m1")
            nc.vector.reduce_max(m1[:], mx[:], axis=mybir.AxisListType.X)
            mt_ps = psC.tile([1, 128], F32, tag="small")
            nc.tensor.transpose(mt_ps[:], m1[:], ident32[:])
            g1 = poolb.tile([1, 1], F32, tag="g1")
            nc.vector.reduce_max(g1[:], mt_ps[:], axis=mybir.AxisListType.X)
            negg = poolb.tile([1, 1], F32, tag="negg")
            nc.vector.tensor_scalar_mul(negg[:], g1[:], -1.0)
            neggb = poolb.tile([128, 1], F32, tag="neggb")
            nc.gpsimd.partition_broadcast(neggb[:], negg[:])

            bexp = poolb.tile([128, P // 128], F32, tag="bexp")
            nc.scalar.activation(bexp[:], mx[:], AF.Exp, bias=neggb[:, 0, None])

            # q2c (unnormalized): sum_p exp(m_p - g) * H[p, :]
            q2c_ps = psF.tile([D, 1], F32, tag="q2c_ps")
            for c in range(P // 128):
                nc.tensor.matmul(q2c_ps[:], lhsT=hn[:, c, :],
                                 rhs=bexp[:, c, None],
                                 start=(c == 0), stop=(c == P // 128 - 1))

            # Zb = sum_p exp(m_p - g)
            zrow_ps = psC.tile([1, P // 128], F32, tag="small")
            nc.tensor.matmul(zrow_ps[:], lhsT=onescol[:], rhs=bexp[:],
                             start=True, stop=True)
            zb = poolb.tile([1, 1], F32, tag="zb")
            nc.vector.reduce_sum(zb[:], zrow_ps[:], axis=mybir.AxisListType.X)
            rzb = poolb.tile([1, 1], F32, tag="rzb")
            nc.vector.reciprocal(rzb[:], zb[:])
            rzbb = poolb.tile([128, 1], F32, tag="rzbb")
            nc.gpsimd.partition_broadcast(rzbb[:], rzb[:])

            q2c = poolb.tile([D, 1], F32, tag="q2c")
            nc.vector.tensor_scalar_mul(q2c[:], q2c_ps[:], rzbb[:, 0, None])

            # pooled columns [d, 5]: [maxH, maxC, maxP, maxHq2c, maxM]
            pooled = poolb.tile([128, 5], F32, tag="pooled")
            nc.vector.reduce_max(pooled[:, 0, None], maxh_c[:],
                                 axis=mybir.AxisListType.X)
            nc.vector.reduce_max(pooled[:, 1, None], maxc_c[:],
                                 axis=mybir.AxisListType.X)
            nc.vector.reduce_max(pooled[:, 2, None], maxp_c[:],
                                 axis=mybir.AxisListType.X)

            # max over valid p of H*q2c from maxH/minH and q2c sign
            nm = poolb.tile([128, 1], F32, tag="nm")
            nc.vector.tensor_reduce(nm[:], minh_c[:], axis=mybir.AxisListType.X,
                                    op=ALU.min)
            t1 = poolb.tile([128, 1], F32, tag="t1")
            nc.vector.tensor_tensor(out=t1[:], in0=q2c[:],
                                    in1=pooled[:, 0, None], op=ALU.mult)
            t2 = poolb.tile([128, 1], F32, tag="t2")
            nc.vector.tensor_tensor(out=t2[:], in0=q2c[:], in1=nm[:], op=ALU.mult)
            nc.vector.tensor_tensor(out=pooled[:, 3, None], in0=t1[:], in1=t2[:],
                                    op=ALU.max)

            # M: fold macc chunks, transpose, reduce over lanes
            mfold = poolb.tile([128, D], F32, tag="mfold")
            nc.vector.reduce_max(
                mfold[:], macc[:].rearrange("l c d -> l d c"),
                axis=mybir.AxisListType.X)
            mt2_ps = psC.tile([D, 128], F32, tag="small")
            nc.tensor.transpose(mt2_ps[:], mfold[:], ident32[:])
            nc.vector.reduce_max(pooled[:, 4, None], mt2_ps[:],
                                 axis=mybir.AxisListType.X)

            # final classifier: out[1,2] = sum_k pooled[:,k]^T @ Wcls[k]
            out_ps = psC.tile([1, 2], F32, tag="small")
            for k in range(5):
                nc.tensor.matmul(out_ps[:], lhsT=pooled[:, k, None],
                                 rhs=wcls_sb[:, k, :],
                                 start=(k == 0), stop=(k == 4))
            out_sb = poolb.tile([1, 2], F32, tag="out_sb")
            nc.scalar.copy(out_sb[:], out_ps[:])
            nc.sync.dma_start(out_ext[b, None, :], out_sb[:])

    nc.compile()
    return nc


_CACHED_NC = None


def _get_program():
    global _CACHED_NC
    if _CACHED_NC is None:
        _CACHED_NC = build_program()
    return _CACHED_NC


def make_in_maps(tensor_H, tensor_U, M, sentence_word_rep, W_attn, W_cls):
    tensor_H = np.ascontiguousarray(np.asarray(tensor_H, dtype=np.float32))
    tensor_U = np.ascontiguousarray(np.asarray(tensor_U, dtype=np.float32))
    M = np.ascontiguousarray(np.asarray(M, dtype=np.float32))
    W_attn = np.ascontiguousarray(np.asarray(W_attn, dtype=np.float32))
    W_cls = np.ascontiguousarray(np.asarray(W_cls, dtype=np.float32))
    swr = np.asarray(sentence_word_rep)

    import ml_dtypes
    bias = np.where(swr == 0, np.float32(NEG), np.float32(0.0)).astype(np.float32)
    bias16 = bias.astype(ml_dtypes.bfloat16)
    M = M.copy()
    M[np.asarray(swr) == 0] = np.float32(NEG)
    bias16p = (-2.0 * bias).astype(ml_dtypes.bfloat16)

    in_maps = []
    for core in range(N_CORES):
        sl = slice(core * B_CORE, (core + 1) * B_CORE)
        in_maps.append({
            "h": tensor_H[sl],
            "m": M[sl],
            "u": tensor_U[sl],
            "w": W_attn,
            "wcls": W_cls,
            "mrow": np.ascontiguousarray(bias[sl]),
            "mrow16": np.ascontiguousarray(bias16[sl]),
            "mrow16p": np.ascontiguousarray(bias16p[sl]),
        })
    return in_maps


def kernel(tensor_H, tensor_U, M, sentence_word_rep, W_attn, W_cls):
    nc = _get_program()
    in_maps = make_in_maps(tensor_H, tensor_U, M, sentence_word_rep,
                           W_attn, W_cls)
    res = run_bass_kernel_spmd(nc, in_maps, list(range(N_CORES)))
    out = np.concatenate([res.results[i]["out"] for i in range(N_CORES)], axis=0)
    return out.astype(np.float32)
